# revision 47
# baseline (speedup 1.0000x reference)
"""BEV->RV scatter-max kernel for 8 Trainium2 NeuronCores.

Strategy: shard by (batch, BEV-quadrant). Each BEV grid quadrant maps to a
disjoint RV column range (phi quadrants), so the 8 cores (2 batches x 4
quadrants) produce disjoint output slabs.

Layout (host, static/data-independent): pixels of each quadrant are grouped by
RV column into segments of SEG_K=2 slots; segments are globally ordered by
their static row-window center so each output row r only touches a contiguous
hull of segments (bounds are compile-time constants, union over quadrants).
Values live in SBUF as fp16 in slot-major layout [P, C, k, j] (k = slot within
segment, j = segment index) so every engine op has a packed innermost dim
(2x fp16 DVE mode).

Device: computes row_high by a 30-plane select on z (planes interleaved across
DVE and Pool, batched-group DMA), builds s=min(l,h), e=max(l,h); then per RV
row: a {0,-60000} additive mask (two 4x tensor_scalar compares + add; the
-120000 double-mask overflows to -inf which is fine for max), one
channel-broadcast masked add over the hull split 22 channels on DVE / 10 on
Pool (Pool supports only add/mult tensor_tensor), and a single pairwise max
fold on DVE, software-pipelined two rows deep so Pool never stalls DVE.
Per-segment maxes are DMA'd raggedly (hull only) per row; the host reduces
segments to columns and assembles the full output.
"""
import math
import sys

sys.path.insert(0, "/opt/trn_rl_repo")

import numpy as np

H_B, W_B = 512, 512
H_R, W_R = 64, 2048
Z_MIN, Z_MAX = -4.0, 2.0
Z_BINS = 30
Z_LOW = -1.73
PHI_MIN, PHI_MAX = -math.pi, math.pi
THETA_MIN, THETA_MAX = math.radians(-25.0), math.radians(3.0)
XMIN, XMAX, YMIN, YMAX = -50.0, 50.0, -50.0, 50.0

C = 32
B = 2
P = 128
SEG_K = 2            # pixels per segment
NEG = np.float16(-60000.0)
BIG = 60000.0        # exactly representable in fp16
NCH_DVE = 22         # channels whose masked add runs on DVE; rest on gpsimd
CH_ROWS = 1          # rows per output DMA chunk

_QUADS = {
    0: (slice(0, 256), slice(0, 256)),
    1: (slice(0, 256), slice(256, 512)),
    2: (slice(256, 512), slice(0, 256)),
    3: (slice(256, 512), slice(256, 512)),
}


def _geometry_f32():
    """Replicates reference._geometry() numpy-f32 semantics exactly."""
    y = np.linspace(YMAX, YMIN, H_B, dtype=np.float32)
    x = np.linspace(XMIN, XMAX, W_B, dtype=np.float32)
    yg, xg = np.meshgrid(y, x, indexing="ij")
    rho = np.sqrt((xg * xg + yg * yg).astype(np.float32)).astype(np.float32)
    phi = np.arctan2(yg, xg)
    theta_low = np.arctan2(np.float32(Z_LOW), rho)
    row_low = np.clip(
        np.rint((THETA_MAX - theta_low) / (THETA_MAX - THETA_MIN) * (H_R - 1)),
        0, H_R - 1,
    ).astype(np.int32)
    col = np.clip(
        np.rint((phi - PHI_MIN) / (PHI_MAX - PHI_MIN) * (W_R - 1)), 0, W_R - 1
    ).astype(np.int32)
    return rho, row_low, col


def _row_high_table(rho_flat):
    """H[z, n]: row_high for each z bin, f32 ops replicating the reference."""
    dz = (Z_MAX - Z_MIN) / Z_BINS
    zc = (np.arange(Z_BINS).astype(np.float32) * np.float32(dz)
          + np.float32(Z_MIN + dz / 2)).astype(np.float32)
    th = np.arctan2(zc[:, None].astype(np.float32), rho_flat[None, :]).astype(np.float32)
    a = (np.float32(THETA_MAX) - th).astype(np.float32)
    b = (a / np.float32(THETA_MAX - THETA_MIN)).astype(np.float32)
    cexpr = (b * np.float32(H_R - 1)).astype(np.float32)
    return np.clip(np.rint(cexpr), 0, H_R - 1).astype(np.int32)  # [30, N]


class _Static:
    pass


_S = None


def _build_static():
    global _S
    if _S is not None:
        return _S
    S = _Static()
    rho, row_low, col = _geometry_f32()

    quads = []
    for q in range(4):
        si, sj = _QUADS[q]
        qcol = col[si, sj].ravel()
        qrho = rho[si, sj].ravel().astype(np.float32)
        qrl = row_low[si, sj].ravel()
        ii, jj = np.meshgrid(np.arange(si.start, si.stop),
                             np.arange(sj.start, sj.stop), indexing="ij")
        qpix = (ii * W_B + jj).ravel()

        Hq = _row_high_table(qrho)
        smin_pix = np.minimum(qrl, Hq.min(0))
        smax_pix = np.maximum(qrl, Hq.max(0))

        # group pixels by column; within column order by static window center
        order = np.lexsort((smin_pix + smax_pix, qcol))
        c0, c1 = int(qcol.min()), int(qcol.max())
        ncols = c1 - c0 + 1
        counts = np.bincount(qcol - c0, minlength=ncols)

        seg_col, slot_src, seg_win = [], [], []
        pos = 0
        for ci in range(ncols):
            k = counts[ci]
            idxs = order[pos:pos + k]
            pos += k
            for off in range(0, k, SEG_K):
                chunk = idxs[off:off + SEG_K]
                seg_col.append(c0 + ci)
                slot_src.append(chunk)
                seg_win.append((smin_pix[chunk].min(), smax_pix[chunk].max()))
        nseg = len(seg_col)
        seg_col = np.asarray(seg_col, np.int32)
        seg_win = np.asarray(seg_win, np.int64)

        # global segment order: by row-window center -> per-row contiguous hull
        gorder = np.argsort(seg_win[:, 0] + seg_win[:, 1], kind="stable")
        seg_col = seg_col[gorder]
        seg_win = seg_win[gorder]
        slot_src = [slot_src[s] for s in gorder]

        Q = _Static()
        Q.nseg = nseg
        Q.seg_col = seg_col
        Q.seg_win = seg_win
        Q.slot_src = slot_src
        Q.qrl = qrl
        Q.Hq = Hq
        Q.qpix = qpix
        quads.append(Q)

    SEG_PP = max(-(-Q.nseg // P) for Q in quads)  # j-slots per partition
    F = SEG_K * SEG_PP
    S.SEG_PP, S.F = SEG_PP, F

    # per-row hulls (j-slot units), union over quadrants
    jA = np.full(H_R, 10 ** 9, np.int64)
    jB = np.full(H_R, -1, np.int64)
    for Q in quads:
        w = Q.seg_win
        for r in range(H_R):
            act = np.flatnonzero((w[:, 0] <= r) & (w[:, 1] >= r))
            if act.size:
                jA[r] = min(jA[r], act.min() // P)
                jB[r] = max(jB[r], act.max() // P)
    assert (jB >= 0).all()
    S.jA, S.jB = jA.astype(int), jB.astype(int)
    S.nj = (jB - jA + 1).astype(int)
    # ragged output offsets (elements per partition): row r block is [C, nj_r]
    offs = np.zeros(H_R + 1, np.int64)
    for r in range(H_R):
        offs[r + 1] = offs[r] + C * S.nj[r]
    S.offs = offs
    S.total_out = int(offs[-1])

    # device tables + host reduce metadata per quadrant
    for Q in quads:
        nseg = Q.nseg
        segs = np.arange(nseg)
        pq = segs % P
        jq = segs // P
        # slot (i) of segment s lives at partition pq, F-index i*SEG_PP + jq
        dst_all, src_all = [], []
        for s, chunk in enumerate(Q.slot_src):
            base = pq[s] * F + jq[s]
            dst_all.append(base + np.arange(len(chunk)) * SEG_PP)
            src_all.append(chunk)
        dst_all = np.concatenate(dst_all).astype(np.int64)
        src_all = np.concatenate(src_all).astype(np.int64)
        Q.dst = dst_all
        Q.qpix_src = Q.qpix[src_all]

        l_tab = np.full(P * F, 127.0, np.float16)
        l_tab[dst_all] = Q.qrl[src_all].astype(np.float16)
        H_tab = np.full((Z_BINS, P * F), 127.0, np.float16)
        H_tab[:, dst_all] = Q.Hq[:, src_all].astype(np.float16)
        Q.l_tab = l_tab.reshape(P, F)
        # partition-major [P, Z_BINS*F] to match the device DMA layout
        Q.H_tab = np.ascontiguousarray(
            H_tab.reshape(Z_BINS, P, F).transpose(1, 0, 2)).reshape(P, Z_BINS * F)

        # host-side per-row reduce metadata: local seg index within hull range
        Q.row_sel, Q.row_starts, Q.row_cols = [], [], []
        for r in range(H_R):
            lo, hi = S.jA[r] * P, (S.jB[r] + 1) * P
            hi_eff = min(hi, nseg)
            if hi_eff <= lo:
                Q.row_sel.append(np.zeros(0, np.int64))
                Q.row_starts.append(np.zeros(0, np.int64))
                Q.row_cols.append(np.zeros(0, np.int64))
                continue
            local = np.arange(lo, hi_eff)
            cols = Q.seg_col[local]
            o = np.argsort(cols, kind="stable")
            sc = cols[o]
            starts = np.flatnonzero(np.r_[True, sc[1:] != sc[:-1]])
            Q.row_sel.append((local - lo)[o])
            Q.row_starts.append(starts)
            Q.row_cols.append(sc[starts])
    S.quads = quads
    _S = S
    return S


_NC = None


def _build_nc():
    global _NC
    if _NC is not None:
        return _NC
    import concourse.bass as bass
    import concourse.bacc as bacc
    import concourse.mybir as mybir
    from concourse.tile import TileContext

    S = _build_static()
    SEG_PP, F = S.SEG_PP, S.F
    f16 = mybir.dt.float16
    Alu = mybir.AluOpType

    nc = bacc.Bacc("TRN2", target_bir_lowering=False, debug=False, num_devices=8)
    # all inputs partition-major so batched DMAs iterate in matching order
    vals = nc.declare_dram_parameter("vals", [P, C * F], f16, isOutput=False)
    tabs = nc.declare_dram_parameter("tabs", [P, 2 * F], f16, isOutput=False)
    htab = nc.declare_dram_parameter("htab", [P, Z_BINS * F], f16, isOutput=False)
    out = nc.declare_dram_parameter("out", [P, S.total_out], f16, isOutput=True)

    Z_DVE = 21  # z planes selected on DVE; rest on Pool

    with TileContext(nc) as tc:
        with tc.tile_pool(name="sb", bufs=1) as pool, \
             tc.tile_pool(name="hplane", bufs=2) as hpool, \
             tc.tile_pool(name="hplane2", bufs=2) as hpool2, \
             tc.tile_pool(name="mbp", bufs=2) as mbpool, \
             tc.tile_pool(name="tmpp", bufs=3) as tmppool, \
             tc.tile_pool(name="outp", bufs=3) as outpool:
            # tables first: the 30-plane select needs them immediately, while
            # the values aren't read until the first row's masked add.
            zl_t = pool.tile([P, 2 * F], f16, tag="zl")
            nc.sync.dma_start(out=zl_t[:], in_=tabs[:, :])
            zb_t = zl_t[:, 0:F]
            l_t = zl_t[:, F:2 * F]
            v_all = pool.tile([P, C * F], f16, tag="vall")

            # 30-plane select of h = H[zbin], split across DVE and Pool
            h_d = pool.tile([P, F], f16, tag="hd")
            h_p = pool.tile([P, F], f16, tag="hp_acc")
            eq_d = pool.tile([P, F], f16, tag="eqd")
            eq_p = pool.tile([P, F], f16, tag="eqp")
            nc.vector.memset(h_d[:], 0.0)
            nc.gpsimd.memset(h_p[:], 0.0)
            # interleave plane ownership so both engines stream concurrently;
            # batch plane DMAs in groups of 5 to amortize per-DMA overhead
            GRP = 5
            n_groups = -(-Z_BINS // GRP)
            vchunk = -(-C // n_groups)
            for g in range(0, Z_BINS, GRP):
                ng = min(GRP, Z_BINS - g)
                hp = hpool.tile([P, GRP * F], f16, tag="hgrp", name="hp")
                nc.sync.dma_start(out=hp[:, :ng * F],
                                  in_=htab[:, g * F:(g + ng) * F])
                # interleave value chunks between plane groups: each chunk is
                # short enough not to starve the select of its next planes
                c0 = (g // GRP) * vchunk
                c1 = min(c0 + vchunk, C)
                if c0 < c1:
                    nc.sync.dma_start(out=v_all[:, c0 * F:c1 * F],
                                      in_=vals[:, c0 * F:c1 * F])
                for k in range(ng):
                    z = g + k
                    on_pool = (z % 10) < 3
                    eng = nc.gpsimd if on_pool else nc.vector
                    eq, acc = (eq_p, h_p) if on_pool else (eq_d, h_d)
                    eng.tensor_scalar(out=eq[:], in0=zb_t[:], scalar1=float(z),
                                      scalar2=None, op0=Alu.is_equal)
                    eng.tensor_tensor(out=eq[:], in0=eq[:],
                                      in1=hp[:, k * F:(k + 1) * F], op=Alu.mult)
                    eng.tensor_tensor(out=acc[:], in0=acc[:], in1=eq[:],
                                      op=Alu.add)

            s_t = pool.tile([P, F], f16, tag="s")
            e_t = pool.tile([P, F], f16, tag="e")
            nc.vector.tensor_tensor(out=h_d[:], in0=h_d[:], in1=h_p[:], op=Alu.add)
            nc.vector.tensor_tensor(out=s_t[:], in0=l_t[:], in1=h_d[:], op=Alu.min)
            nc.vector.tensor_tensor(out=e_t[:], in0=l_t[:], in1=h_d[:], op=Alu.max)

            NCH_P = C - NCH_DVE
            njmax = int(S.nj.max())
            ch_max = max(int(S.offs[min(r0 + CH_ROWS, H_R)] - S.offs[r0])
                         for r0 in range(0, H_R, CH_ROWS))

            def hull_ap(tile, joff, nj, extra=0):
                # 3D view [P, SEG_K, nj] of a [P, F]-layout tile at j-offset
                t = tile[:]
                return bass.AP(t.tensor, t.offset + joff + extra,
                               [t.ap[0], [SEG_PP, SEG_K], [1, nj]])

            # Software-pipelined: row r's masked adds (DVE low channels, Pool
            # high channels) are emitted one iteration before row r's folds
            # (all on DVE), so Pool's adds never stall DVE.
            chunks = {}   # chunk index -> (tile, base_off, n_done)
            pending = []  # (r, tmp_tile)

            def emit_folds(r, tmp):
                jA, nj = int(S.jA[r]), int(S.nj[r])
                ci = r // CH_ROWS
                if ci not in chunks:
                    ch_tile = outpool.tile([P, ch_max], f16, tag="out", name="ch_t")
                    chunks[ci] = [ch_tile, int(S.offs[ci * CH_ROWS]), 0]
                ch_t, base, _ = chunks[ci]
                row_off = int(S.offs[r]) - base
                # single fold: [C, 2, nj] -> [C, nj] straight into the chunk
                tt = tmp[:]
                i0 = bass.AP(tt.tensor, tt.offset,
                             [tt.ap[0], [2 * nj, C], [1, nj]])
                i1 = bass.AP(tt.tensor, tt.offset + nj,
                             [tt.ap[0], [2 * nj, C], [1, nj]])
                cc = ch_t[:]
                o_ap = bass.AP(cc.tensor, cc.offset + row_off,
                               [cc.ap[0], [nj, C], [1, nj]])
                nc.vector.tensor_tensor(out=o_ap, in0=i0, in1=i1, op=Alu.max)
                chunks[ci][2] += 1
                if chunks[ci][2] == min(CH_ROWS, H_R - ci * CH_ROWS):
                    ch_elems = int(S.offs[min((ci + 1) * CH_ROWS, H_R)]) - base
                    nc.sync.dma_start(
                        out=out[:, base:base + ch_elems], in_=ch_t[:, :ch_elems])

            for r in range(H_R):
                jA, nj = int(S.jA[r]), int(S.nj[r])
                fr = float(r)
                a_t = mbpool.tile([P, F], f16, tag="a")
                b_t = mbpool.tile([P, F], f16, tag="b")
                mb_t = mbpool.tile([P, F], f16, tag="mb", bufs=3)
                # a = (s > r) * -BIG ; b = (e < r) * -BIG ; mb = a + b
                nc.vector.tensor_scalar(
                    out=hull_ap(a_t, jA, nj), in0=hull_ap(s_t, jA, nj),
                    scalar1=fr, scalar2=-BIG, op0=Alu.is_gt, op1=Alu.mult)
                nc.vector.tensor_scalar(
                    out=hull_ap(b_t, jA, nj), in0=hull_ap(e_t, jA, nj),
                    scalar1=fr, scalar2=-BIG, op0=Alu.is_lt, op1=Alu.mult)
                nc.vector.tensor_tensor(
                    out=hull_ap(mb_t, jA, nj), in0=hull_ap(a_t, jA, nj),
                    in1=hull_ap(b_t, jA, nj), op=Alu.add)

                tmp = tmppool.tile([P, C * SEG_K * njmax], f16, tag="tmp")
                for eng, ch0, nch in ((nc.vector, 0, NCH_DVE),
                                      (nc.gpsimd, NCH_DVE, NCH_P)):
                    vv = v_all[:]
                    v_ap = bass.AP(
                        vv.tensor, vv.offset + ch0 * F + jA,
                        [vv.ap[0], [F, nch], [SEG_PP, SEG_K], [1, nj]])
                    mm = mb_t[:]
                    mb_ap = bass.AP(
                        mm.tensor, mm.offset + jA,
                        [mm.ap[0], [0, nch], [SEG_PP, SEG_K], [1, nj]])
                    tt = tmp[:]
                    t_ap = bass.AP(
                        tt.tensor, tt.offset + ch0 * SEG_K * nj,
                        [tt.ap[0], [SEG_K * nj, nch], [nj, SEG_K], [1, nj]])
                    eng.tensor_tensor(out=t_ap, in0=v_ap, in1=mb_ap, op=Alu.add)
                pending.append((r, tmp))
                if len(pending) > 2:
                    emit_folds(*pending.pop(0))
            while pending:
                emit_folds(*pending.pop(0))
    nc.compile()
    _NC = nc
    return nc


def kernel(bev_feat, bev_z_bin):
    from concourse.bass_utils import run_bass_kernel_spmd

    S = _build_static()
    nc = _build_nc()
    F = S.F
    bev_feat = np.asarray(bev_feat, dtype=np.float32)
    bev_z_bin = np.asarray(bev_z_bin, dtype=np.int32)

    in_maps = []
    metas = []
    for core in range(8):
        b, q = core // 4, core % 4
        Q = S.quads[q]
        flat = bev_feat[b].reshape(C, H_B * W_B)
        v = np.full((C, P * F), NEG, np.float16)
        v[:, Q.dst] = flat[:, Q.qpix_src].astype(np.float16)
        zflat = bev_z_bin[b, 0].reshape(H_B * W_B)
        z = np.zeros(P * F, np.float16)
        z[Q.dst] = zflat[Q.qpix_src].astype(np.float16)
        in_maps.append({
            "vals": np.ascontiguousarray(
                v.reshape(C, P, F).transpose(1, 0, 2)).reshape(P, C * F),
            "tabs": np.concatenate([z.reshape(P, F), Q.l_tab], axis=1),
            "htab": Q.H_tab,
        })
        metas.append((b, q))

    res = run_bass_kernel_spmd(nc, in_maps, list(range(8)))

    outp = np.zeros((B, C, H_R, W_R), np.float32)
    for core, (b, q) in enumerate(metas):
        Q = S.quads[q]
        o = np.asarray(res.results[core]["out"]).astype(np.float32)  # [P, total]
        for r in range(H_R):
            sel = Q.row_sel[r]
            if sel.size == 0:
                continue
            nj = int(S.nj[r])
            blk = o[:, int(S.offs[r]):int(S.offs[r]) + C * nj].reshape(P, C, nj)
            # seg local index within hull = j*P + p -> arr[:, j*P+p]
            arr = blk.transpose(1, 2, 0).reshape(C, nj * P)
            red = np.maximum.reduceat(arr[:, sel], Q.row_starts[r], axis=1)
            good = red > -30000.0
            outp[b][:, r, :][:, Q.row_cols[r]] = np.where(good, red, 0.0)
    return outp
